# revision 39
# baseline (speedup 1.0000x reference)
"""GCN layer kernel for Trainium2 (Bass/Tile), data-parallel over batch.

Reference computation (per batch element):
    deg = A.sum(-1); d = deg ** -0.5
    t   = X @ W.T + b
    out = relu(diag(d) @ A @ diag(d) @ t)

Per-core mapping (8 cores, one batch element each). Host-side staging is
layout/dtype only (transposes + bf16 rounding, the same rounding the device
matmul path would apply); all model arithmetic (degree, normalization,
matmuls, bias, relu) runs on device:
  - A is staged twice in bf16: AT (transposed, the matmul stationary) and
    AN (natural, for the on-device degree row-sums). Streaming over the
    contraction index k, AT row-tile k provides the stationary chunks for
    ALL 16 output tiles, so each step runs a uniform batch of 16 products
    (k, mu) — no triangular schedule and no on-device transposes.
  - deg row-sums split DVE/ACT (accum_out) from AN tiles; d = sqrt(1/deg).
  - t = X @ W.T + b in bf16 from host-staged XT/WT, two chains per PSUM
    bank (8 wide drains split ACT/DVE); the bias is folded in as a K=1
    ones x b product initializing each group. Pairs 0-1 run in the head
    (doubling as PE warm-up for the HAM clock gate, topped up by a few
    identity matmuls); pairs 2-7 interleave into the first stream steps.
    y[k] = d[k] * t[k] rounded to bf16 by ACT.
  - All 16 output chains accumulate in PSUM f32 simultaneously, packed
    2-per-bank across all 8 banks (half-bank sharing: the bank's first
    matmul uses start=True, which marks the whole 2KB zero-region
    pending-zero; the partner chain's first matmul uses start=False and
    overwrites its still-pending half; the bank's last matmul carries
    stop=True). Chains 12..15 live in the banks that host mm1 first, so
    their products lag LAG_TR steps behind the stream.
  - Drain: relu(d * psum) split ACT/DVE, stores batched 4 row-tiles per
    dispatch alternating the sync (HWDGE) and gpsimd (SWDGE) queues.
"""

from contextlib import ExitStack

import numpy as np
import ml_dtypes

import concourse.bacc as bacc
import concourse.mybir as mybir
import concourse.tile as tile
from concourse.bass_utils import run_bass_kernel_spmd
from concourse.masks import make_identity

B = 8
N = 2048
F = 256
P = 128
NT = N // P  # 16 row tiles
FT = F // P  # 2 feature chunks
F32 = mybir.dt.float32
BF16 = mybir.dt.bfloat16
COPY = mybir.ActivationFunctionType.Copy
RELU = mybir.ActivationFunctionType.Relu
PF = 5  # A tiles (of each kind) prefetched ahead
STORE_BATCH = 2
WARMUP_MMS = 70  # identity matmuls leading the PE queue: HAM warm-up
LAG_TR = 4  # steps by which chains 12..15 lag (their banks host mm1 first)
RED_AHEAD = 2  # degree reduces run this many steps ahead of their y


def _emit(ctx: ExitStack, tc: tile.TileContext, AT, AN, XT, WTB, BIASB, OUT):
    nc = tc.nc

    const = ctx.enter_context(tc.tile_pool(name="const", bufs=1))
    at_stage = ctx.enter_context(tc.tile_pool(name="at_stage", bufs=6))
    an_stage = ctx.enter_context(tc.tile_pool(name="an_stage", bufs=4))
    scr = ctx.enter_context(tc.tile_pool(name="scr", bufs=4))
    outstage = ctx.enter_context(tc.tile_pool(name="outstage", bufs=8))
    psum_acc = ctx.enter_context(tc.tile_pool(name="psum_acc", bufs=6, space="PSUM"))
    psum_tr = ctx.enter_context(tc.tile_pool(name="psum_tr", bufs=2, space="PSUM"))

    # ---- head DMA, one queue, critical-path order (XT feeds mm1 first) ----
    xt_sb = const.tile([P, FT * N], BF16, tag="xt")
    nc.sync.dma_start(
        out=xt_sb[:, :].rearrange("p (c n) -> p c n", c=FT),
        in_=XT.rearrange("(c p) n -> p c n", p=P),
    )
    wt_sb = const.tile([P, FT * F], BF16, tag="wt")
    nc.sync.dma_start(
        out=wt_sb[:, :].rearrange("p (c f) -> p c f", c=FT),
        in_=WTB.rearrange("(c p) f -> p c f", p=P),
    )
    b_bf = const.tile([1, F], BF16, tag="bbf")
    nc.sync.dma_start(out=b_bf[:, :], in_=BIASB[:, :])

    at_tiles = {}
    an_tiles = {}

    def emit_load_pair(pr):
        an_p = an_stage.tile([P, 2 * N], BF16, tag="an", name=f"an_{pr}")
        at_p = at_stage.tile([P, 2 * N], BF16, tag="at", name=f"at_{pr}")
        if pr == 0:
            # first pair as 4 single-tile loads: earliest possible first data
            for h in range(2):
                nc.sync.dma_start(
                    out=an_p[:, h * N : (h + 1) * N], in_=AN[h * P : (h + 1) * P, :]
                )
                nc.sync.dma_start(
                    out=at_p[:, h * N : (h + 1) * N], in_=AT[h * P : (h + 1) * P, :]
                )
        else:
            nc.sync.dma_start(
                out=an_p[:, :].rearrange("p (t n) -> p t n", t=2),
                in_=AN.rearrange("(t p) n -> p t n", p=P)[:, 2 * pr : 2 * pr + 2, :],
            )
            nc.sync.dma_start(
                out=at_p[:, :].rearrange("p (t n) -> p t n", t=2),
                in_=AT.rearrange("(t p) n -> p t n", p=P)[:, 2 * pr : 2 * pr + 2, :],
            )
        for h in range(2):
            an_tiles[2 * pr + h] = an_p[:, h * N : (h + 1) * N]
            at_tiles[2 * pr + h] = at_p[:, h * N : (h + 1) * N]

    for pr in range(3):
        emit_load_pair(pr)

    ones_bf = const.tile([1, P], BF16, tag="ones")
    nc.vector.memset(ones_bf[:, :], 1.0)
    ident = const.tile([P, P], BF16, tag="ident")
    make_identity(nc, ident[:, :])

    dega = const.tile([P, NT], F32, tag="dega")
    degb = const.tile([P, NT], F32, tag="degb")
    rec = const.tile([P, NT], F32, tag="rec")
    dinv = const.tile([P, NT], F32, tag="dinv")
    t_big = const.tile([P, NT * F], F32, tag="t")
    y_big = const.tile([P, NT * F], BF16, tag="y")

    # warm-up leads the PE queue: it runs while the head loads are in flight,
    # trips the HAM un-throttle, and ends roughly when mm1's inputs land
    warm = psum_acc.tile([P, 2 * F], F32, tag="acc", name="warm")
    for _ in range(WARMUP_MMS):
        nc.tensor.matmul(
            warm[:, 0:P], ident[:, :], ident[:, :], start=True, stop=True
        )

    def emit_reduce(k, an_t):
        # degree row-sums, alternating DVE (even k) / ACT (odd k); the single
        # reader also frees the AN buffer promptly for the next load
        sc = scr.tile([P, N], BF16, tag="sc", name=f"sc_{k}")
        if k % 2 == 0:
            nc.vector.tensor_scalar(
                out=sc[:, :],
                in0=an_t[:, :],
                scalar1=0.0,
                scalar2=None,
                op0=mybir.AluOpType.add,
                op1=mybir.AluOpType.add,
                accum_out=dega[:, k : k + 1],
            )
        else:
            nc.scalar.activation(
                sc[:, :], an_t[:, :], COPY, accum_out=dega[:, k : k + 1]
            )
        nc.vector.reciprocal(rec[:, k : k + 1], dega[:, k : k + 1])

    # ---- mm1 pair-chains (two t-tiles per PSUM bank) ----
    tpp = {}

    def emit_mm1_pair(pj):
        tpp[pj] = psum_tr.tile([P, 2 * F], F32, tag="tr", name=f"tpp_{pj}")
        for jj in range(2):
            j = 2 * pj + jj
            reg = tpp[pj][:, jj * F : (jj + 1) * F]
            nc.tensor.matmul(
                reg, ones_bf[:, :], b_bf[:, :], start=(jj == 0), stop=False
            )
            for phi in range(FT):
                nc.tensor.matmul(
                    reg,
                    xt_sb[:, phi * N + j * P : phi * N + (j + 1) * P],
                    wt_sb[:, phi * F : (phi + 1) * F],
                    start=False,
                    stop=(jj == 1 and phi == FT - 1),
                )

    def emit_pair_drain(pj):
        dst = t_big[:, 2 * pj * F : (2 * pj + 2) * F]
        if pj % 2 == 0:
            nc.scalar.copy(dst, tpp[pj][:, :])
        else:
            nc.vector.tensor_copy(dst, tpp[pj][:, :])

    # prime the reduce-ahead pipeline before any mm1 drains can block a queue
    # (both on DVE so ACT's FIFO stays clear for the first sqrt/y)
    for kk in (0, 1):
        scp = scr.tile([P, N], BF16, tag="sc", name=f"sc_p{kk}")
        nc.vector.tensor_scalar(
            out=scp[:, :], in0=an_tiles[kk][:, :], scalar1=0.0, scalar2=None,
            op0=mybir.AluOpType.add, op1=mybir.AluOpType.add,
            accum_out=dega[:, kk : kk + 1],
        )
        nc.vector.reciprocal(rec[:, kk : kk + 1], dega[:, kk : kk + 1])

    emit_mm1_pair(0)
    emit_mm1_pair(1)
    emit_pair_drain(0)

    # ---- all 16 accumulation chains, 2 per bank ----
    acc_banks = [
        psum_acc.tile([P, 2 * F], F32, tag="acc", name=f"accbank_{b_}")
        for b_ in range(6)
    ]
    cbank = {}  # allocated after the last mm1 pair rotates through

    def acc_region(mu):
        half = (mu % 2) * F
        if mu < 12:
            return acc_banks[mu // 2][:, half : half + F]
        return cbank[12 if mu < 14 else 14][:, half : half + F]

    ostiles = {}

    def emit_drain(mu):
        bi = mu // STORE_BATCH
        if bi not in ostiles:
            ostiles[bi] = outstage.tile(
                [P, STORE_BATCH * F], F32, tag="os", name=f"os_{bi}"
            )
        j = mu % STORE_BATCH
        dst = ostiles[bi][:, j * F : (j + 1) * F]
        if mu % 2 == 0:
            nc.scalar.activation(
                dst, acc_region(mu), RELU, scale=dinv[:, mu : mu + 1]
            )
        else:
            nc.vector.tensor_scalar(
                out=dst,
                in0=acc_region(mu),
                scalar1=dinv[:, mu : mu + 1],
                scalar2=0.0,
                op0=mybir.AluOpType.mult,
                op1=mybir.AluOpType.max,
            )
        if j == STORE_BATCH - 1:
            lo = bi * STORE_BATCH
            q = nc.sync if (bi % 2 == 0) else nc.gpsimd
            q.dma_start(
                out=OUT.rearrange("(m p) f -> p m f", p=P)[:, lo : lo + STORE_BATCH, :],
                in_=ostiles[bi][:, :].rearrange("p (m f) -> p m f", m=STORE_BATCH),
            )

    def emit_products(k, mus):
        for mu in mus:
            nc.tensor.matmul(
                acc_region(mu),
                at_tiles[k][:, mu * P : (mu + 1) * P],
                y_big[:, k * F : (k + 1) * F],
                start=(k == 0 and mu % 2 == 0),
                stop=(k == NT - 1 and mu % 2 == 1),
            )

    # ---- stream over the contraction index k ----
    for k in range(NT):
        if k % 2 == 0 and 3 + k // 2 < NT // 2:
            emit_load_pair(3 + k // 2)
        nc.scalar.sqrt(dinv[:, k : k + 1], rec[:, k : k + 1])
        nc.scalar.activation(
            y_big[:, k * F : (k + 1) * F],
            t_big[:, k * F : (k + 1) * F],
            COPY,
            scale=dinv[:, k : k + 1],
        )
        # remaining mm1 pairs ride the first stream steps (PE fill-in)
        if k < 3:
            emit_mm1_pair(2 * k + 2)
            emit_mm1_pair(2 * k + 3)
        if k == 3:
            cbank[12] = psum_tr.tile([P, 2 * F], F32, tag="tr", name="cbank12")
            cbank[14] = psum_tr.tile([P, 2 * F], F32, tag="tr", name="cbank14")
        emit_products(k, range(12))
        if k >= LAG_TR:
            emit_products(k - LAG_TR, range(12, NT))
        # reduce for step k+RED_AHEAD runs now, behind this step's sqrt/y,
        # so y latency never includes a reduce
        ka = k + RED_AHEAD
        if ka < NT:
            emit_reduce(ka, an_tiles.pop(ka))
        # mm1 pair drains trail their chains closely so the tr-bank rotation
        # (and the cbank allocs behind it) never blocks the PE for long
        for pj in {1: (1, 2), 2: (3, 4), 3: (5, 6), 4: (7,)}.get(k, ()):
            emit_pair_drain(pj)

    for k in range(NT - LAG_TR, NT):
        emit_products(k, range(12, NT))

    # ---- tail: relu(d * acc) and batched stores ----
    for mu in range(NT):
        emit_drain(mu)


_cached_nc = None


def _build():
    nc = bacc.Bacc("TRN2", target_bir_lowering=False, debug=False)
    AT = nc.dram_tensor("at", [N, N], BF16, kind="ExternalInput").ap()
    AN = nc.dram_tensor("an", [N, N], BF16, kind="ExternalInput").ap()
    XT = nc.dram_tensor("xt", [F, N], BF16, kind="ExternalInput").ap()
    WTB = nc.dram_tensor("wtb", [F, F], BF16, kind="ExternalInput").ap()
    BIASB = nc.dram_tensor("biasb", [1, F], BF16, kind="ExternalInput").ap()
    OUT = nc.dram_tensor("out", [N, F], F32, kind="ExternalOutput").ap()
    with tile.TileContext(nc) as tc:
        with ExitStack() as ctx:
            _emit(ctx, tc, AT, AN, XT, WTB, BIASB, OUT)
    nc.compile()
    return nc


def get_nc():
    global _cached_nc
    if _cached_nc is None:
        _cached_nc = _build()
    return _cached_nc


def make_in_maps(node_features, adj_matrix, W, b):
    bf16 = ml_dtypes.bfloat16
    node_features = np.asarray(node_features, dtype=np.float32)
    adj_matrix = np.asarray(adj_matrix, dtype=np.float32)
    an = adj_matrix.astype(bf16)  # [B, N, N] natural
    at = np.ascontiguousarray(an.transpose(0, 2, 1))  # [B, N, N] transposed
    xt = np.ascontiguousarray(
        node_features.astype(bf16).transpose(0, 2, 1)
    )  # [B, F, N]
    wtb = np.ascontiguousarray(np.asarray(W, dtype=np.float32).T.astype(bf16))
    biasb = np.ascontiguousarray(
        np.asarray(b, dtype=np.float32).reshape(1, F).astype(bf16)
    )
    return [
        {
            "at": np.ascontiguousarray(at[c]),
            "an": np.ascontiguousarray(an[c]),
            "xt": xt[c],
            "wtb": wtb,
            "biasb": biasb,
        }
        for c in range(B)
    ]


def kernel(node_features, adj_matrix, W, b):
    nc = get_nc()
    in_maps = make_in_maps(node_features, adj_matrix, W, b)
    res = run_bass_kernel_spmd(nc, in_maps, core_ids=list(range(B)))
    return np.stack([r["out"] for r in res.results], axis=0)


# revision 40
# speedup vs baseline: 1.0939x; 1.0939x over previous
"""GCN layer kernel for Trainium2 (Bass/Tile), data-parallel over batch.

Reference computation (per batch element):
    deg = A.sum(-1); d = deg ** -0.5
    t   = X @ W.T + b
    out = relu(diag(d) @ A @ diag(d) @ t)

Per-core mapping (8 cores, one batch element each). Host-side staging is
layout/dtype only (transposes + bf16 rounding, the same rounding the device
matmul path would apply); all model arithmetic (degree, normalization,
matmuls, bias, relu) runs on device:
  - A is staged twice in bf16: AT (transposed, the matmul stationary) and
    AN (natural, for the on-device degree row-sums). Streaming over the
    contraction index k, AT row-tile k provides the stationary chunks for
    ALL 16 output tiles, so each step runs a uniform batch of 16 products
    (k, mu) — no triangular schedule and no on-device transposes.
  - deg row-sums split DVE/ACT (accum_out) from AN tiles; d = sqrt(1/deg).
  - t = X @ W.T + b in bf16 from host-staged XT/WT, two chains per PSUM
    bank (8 wide drains split ACT/DVE); the bias is folded in as a K=1
    ones x b product initializing each group. Pairs 0-1 run in the head
    (doubling as PE warm-up for the HAM clock gate, topped up by a few
    identity matmuls); pairs 2-7 interleave into the first stream steps.
    y[k] = d[k] * t[k] rounded to bf16 by ACT.
  - All 16 output chains accumulate in PSUM f32 simultaneously, packed
    2-per-bank across all 8 banks (half-bank sharing: the bank's first
    matmul uses start=True, which marks the whole 2KB zero-region
    pending-zero; the partner chain's first matmul uses start=False and
    overwrites its still-pending half; the bank's last matmul carries
    stop=True). Chains 12..15 live in the banks that host mm1 first, so
    their products lag LAG_TR steps behind the stream.
  - Drain: relu(d * psum) split ACT/DVE, stores batched 4 row-tiles per
    dispatch alternating the sync (HWDGE) and gpsimd (SWDGE) queues.
"""

from contextlib import ExitStack

import numpy as np
import ml_dtypes

import concourse.bacc as bacc
import concourse.mybir as mybir
import concourse.tile as tile
from concourse.bass_utils import run_bass_kernel_spmd
from concourse.masks import make_identity

B = 8
N = 2048
F = 256
P = 128
NT = N // P  # 16 row tiles
FT = F // P  # 2 feature chunks
F32 = mybir.dt.float32
BF16 = mybir.dt.bfloat16
COPY = mybir.ActivationFunctionType.Copy
RELU = mybir.ActivationFunctionType.Relu
PF = 5  # A tiles (of each kind) prefetched ahead
STORE_BATCH = 4
WARMUP_MMS = 70  # identity matmuls leading the PE queue: HAM warm-up
LAG_TR = 4  # steps by which chains 12..15 lag (their banks host mm1 first)
RED_AHEAD = 2  # degree reduces run this many steps ahead of their y


def _emit(ctx: ExitStack, tc: tile.TileContext, AT, AN, XT, WTB, BIASB, OUT):
    nc = tc.nc

    const = ctx.enter_context(tc.tile_pool(name="const", bufs=1))
    at_stage = ctx.enter_context(tc.tile_pool(name="at_stage", bufs=6))
    an_stage = ctx.enter_context(tc.tile_pool(name="an_stage", bufs=4))
    scr = ctx.enter_context(tc.tile_pool(name="scr", bufs=4))
    outstage = ctx.enter_context(tc.tile_pool(name="outstage", bufs=8))
    psum_acc = ctx.enter_context(tc.tile_pool(name="psum_acc", bufs=6, space="PSUM"))
    psum_tr = ctx.enter_context(tc.tile_pool(name="psum_tr", bufs=2, space="PSUM"))

    # ---- head DMA, one queue, critical-path order (XT feeds mm1 first) ----
    xt_sb = const.tile([P, FT * N], BF16, tag="xt")
    nc.sync.dma_start(
        out=xt_sb[:, :].rearrange("p (c n) -> p c n", c=FT),
        in_=XT.rearrange("(c p) n -> p c n", p=P),
    )
    wt_sb = const.tile([P, FT * F], BF16, tag="wt")
    nc.sync.dma_start(
        out=wt_sb[:, :].rearrange("p (c f) -> p c f", c=FT),
        in_=WTB.rearrange("(c p) f -> p c f", p=P),
    )
    b_bf = const.tile([1, F], BF16, tag="bbf")
    nc.sync.dma_start(out=b_bf[:, :], in_=BIASB[:, :])

    at_tiles = {}
    an_tiles = {}

    def emit_load_pair(pr):
        an_p = an_stage.tile([P, 2 * N], BF16, tag="an", name=f"an_{pr}")
        at_p = at_stage.tile([P, 2 * N], BF16, tag="at", name=f"at_{pr}")
        if pr == 0:
            # first pair as 4 single-tile loads: earliest possible first data
            for h in range(2):
                nc.sync.dma_start(
                    out=an_p[:, h * N : (h + 1) * N], in_=AN[h * P : (h + 1) * P, :]
                )
                nc.sync.dma_start(
                    out=at_p[:, h * N : (h + 1) * N], in_=AT[h * P : (h + 1) * P, :]
                )
        else:
            nc.sync.dma_start(
                out=an_p[:, :].rearrange("p (t n) -> p t n", t=2),
                in_=AN.rearrange("(t p) n -> p t n", p=P)[:, 2 * pr : 2 * pr + 2, :],
            )
            nc.sync.dma_start(
                out=at_p[:, :].rearrange("p (t n) -> p t n", t=2),
                in_=AT.rearrange("(t p) n -> p t n", p=P)[:, 2 * pr : 2 * pr + 2, :],
            )
        for h in range(2):
            an_tiles[2 * pr + h] = an_p[:, h * N : (h + 1) * N]
            at_tiles[2 * pr + h] = at_p[:, h * N : (h + 1) * N]

    for pr in range(3):
        emit_load_pair(pr)

    ones_bf = const.tile([1, P], BF16, tag="ones")
    nc.vector.memset(ones_bf[:, :], 1.0)
    ident = const.tile([P, P], BF16, tag="ident")
    make_identity(nc, ident[:, :])

    dega = const.tile([P, NT], F32, tag="dega")
    degb = const.tile([P, NT], F32, tag="degb")
    rec = const.tile([P, NT], F32, tag="rec")
    dinv = const.tile([P, NT], F32, tag="dinv")
    t_big = const.tile([P, NT * F], F32, tag="t")
    y_big = const.tile([P, NT * F], BF16, tag="y")

    # warm-up leads the PE queue: it runs while the head loads are in flight,
    # trips the HAM un-throttle, and ends roughly when mm1's inputs land
    warm = psum_acc.tile([P, 2 * F], F32, tag="acc", name="warm")
    for _ in range(WARMUP_MMS):
        nc.tensor.matmul(
            warm[:, 0:P], ident[:, :], ident[:, :], start=True, stop=True
        )

    def emit_reduce(k, an_t):
        # degree row-sums, alternating DVE (even k) / ACT (odd k); the single
        # reader also frees the AN buffer promptly for the next load
        sc = scr.tile([P, N], BF16, tag="sc", name=f"sc_{k}")
        if k % 2 == 0:
            nc.vector.tensor_scalar(
                out=sc[:, :],
                in0=an_t[:, :],
                scalar1=0.0,
                scalar2=None,
                op0=mybir.AluOpType.add,
                op1=mybir.AluOpType.add,
                accum_out=dega[:, k : k + 1],
            )
        else:
            nc.scalar.activation(
                sc[:, :], an_t[:, :], COPY, accum_out=dega[:, k : k + 1]
            )
        nc.vector.reciprocal(rec[:, k : k + 1], dega[:, k : k + 1])

    # ---- mm1 pair-chains (two t-tiles per PSUM bank) ----
    tpp = {}

    def emit_mm1_pair(pj):
        tpp[pj] = psum_tr.tile([P, 2 * F], F32, tag="tr", name=f"tpp_{pj}")
        for jj in range(2):
            j = 2 * pj + jj
            reg = tpp[pj][:, jj * F : (jj + 1) * F]
            nc.tensor.matmul(
                reg, ones_bf[:, :], b_bf[:, :], start=(jj == 0), stop=False
            )
            for phi in range(FT):
                nc.tensor.matmul(
                    reg,
                    xt_sb[:, phi * N + j * P : phi * N + (j + 1) * P],
                    wt_sb[:, phi * F : (phi + 1) * F],
                    start=False,
                    stop=(jj == 1 and phi == FT - 1),
                )

    def emit_pair_drain(pj):
        dst = t_big[:, 2 * pj * F : (2 * pj + 2) * F]
        if pj % 2 == 0:
            nc.scalar.copy(dst, tpp[pj][:, :])
        else:
            nc.vector.tensor_copy(dst, tpp[pj][:, :])

    # prime the reduce-ahead pipeline before any mm1 drains can block a queue
    # (both on DVE so ACT's FIFO stays clear for the first sqrt/y)
    for kk in (0, 1):
        scp = scr.tile([P, N], BF16, tag="sc", name=f"sc_p{kk}")
        nc.vector.tensor_scalar(
            out=scp[:, :], in0=an_tiles[kk][:, :], scalar1=0.0, scalar2=None,
            op0=mybir.AluOpType.add, op1=mybir.AluOpType.add,
            accum_out=dega[:, kk : kk + 1],
        )
        nc.vector.reciprocal(rec[:, kk : kk + 1], dega[:, kk : kk + 1])

    emit_mm1_pair(0)
    emit_mm1_pair(1)
    emit_pair_drain(0)

    # ---- all 16 accumulation chains, 2 per bank ----
    acc_banks = [
        psum_acc.tile([P, 2 * F], F32, tag="acc", name=f"accbank_{b_}")
        for b_ in range(6)
    ]
    cbank = {}  # allocated after the last mm1 pair rotates through

    def acc_region(mu):
        half = (mu % 2) * F
        if mu < 12:
            return acc_banks[mu // 2][:, half : half + F]
        return cbank[12 if mu < 14 else 14][:, half : half + F]

    ostiles = {}

    def emit_drain(mu):
        bi = mu // STORE_BATCH
        if bi not in ostiles:
            ostiles[bi] = outstage.tile(
                [P, STORE_BATCH * F], F32, tag="os", name=f"os_{bi}"
            )
        j = mu % STORE_BATCH
        dst = ostiles[bi][:, j * F : (j + 1) * F]
        if mu % 2 == 0:
            nc.scalar.activation(
                dst, acc_region(mu), RELU, scale=dinv[:, mu : mu + 1]
            )
        else:
            nc.vector.tensor_scalar(
                out=dst,
                in0=acc_region(mu),
                scalar1=dinv[:, mu : mu + 1],
                scalar2=0.0,
                op0=mybir.AluOpType.mult,
                op1=mybir.AluOpType.max,
            )
        if j == STORE_BATCH - 1:
            lo = bi * STORE_BATCH
            q = nc.sync if (bi % 2 == 0) else nc.gpsimd
            q.dma_start(
                out=OUT.rearrange("(m p) f -> p m f", p=P)[:, lo : lo + STORE_BATCH, :],
                in_=ostiles[bi][:, :].rearrange("p (m f) -> p m f", m=STORE_BATCH),
            )

    def emit_products(k, mus):
        for mu in mus:
            nc.tensor.matmul(
                acc_region(mu),
                at_tiles[k][:, mu * P : (mu + 1) * P],
                y_big[:, k * F : (k + 1) * F],
                start=(k == 0 and mu % 2 == 0),
                stop=(k == NT - 1 and mu % 2 == 1),
            )

    # ---- stream over the contraction index k ----
    for k in range(NT):
        if k % 2 == 0 and 3 + k // 2 < NT // 2:
            emit_load_pair(3 + k // 2)
        nc.scalar.sqrt(dinv[:, k : k + 1], rec[:, k : k + 1])
        nc.scalar.activation(
            y_big[:, k * F : (k + 1) * F],
            t_big[:, k * F : (k + 1) * F],
            COPY,
            scale=dinv[:, k : k + 1],
        )
        # remaining mm1 pairs ride the first stream steps (PE fill-in)
        if k < 3:
            emit_mm1_pair(2 * k + 2)
            emit_mm1_pair(2 * k + 3)
        if k == 3:
            cbank[12] = psum_tr.tile([P, 2 * F], F32, tag="tr", name="cbank12")
            cbank[14] = psum_tr.tile([P, 2 * F], F32, tag="tr", name="cbank14")
        emit_products(k, range(12))
        if k >= LAG_TR:
            emit_products(k - LAG_TR, range(12, NT))
        # reduce for step k+RED_AHEAD runs now, behind this step's sqrt/y,
        # so y latency never includes a reduce
        ka = k + RED_AHEAD
        if ka < NT:
            emit_reduce(ka, an_tiles.pop(ka))
        # mm1 pair drains trail their chains closely so the tr-bank rotation
        # (and the cbank allocs behind it) never blocks the PE for long
        for pj in {1: (1, 2), 2: (3, 4), 3: (5, 6), 4: (7,)}.get(k, ()):
            emit_pair_drain(pj)

    for k in range(NT - LAG_TR, NT):
        emit_products(k, range(12, NT))

    # ---- tail: relu(d * acc) and batched stores ----
    for mu in range(NT):
        emit_drain(mu)


_cached_nc = None


def _build():
    nc = bacc.Bacc("TRN2", target_bir_lowering=False, debug=False)
    AT = nc.dram_tensor("at", [N, N], BF16, kind="ExternalInput").ap()
    AN = nc.dram_tensor("an", [N, N], BF16, kind="ExternalInput").ap()
    XT = nc.dram_tensor("xt", [F, N], BF16, kind="ExternalInput").ap()
    WTB = nc.dram_tensor("wtb", [F, F], BF16, kind="ExternalInput").ap()
    BIASB = nc.dram_tensor("biasb", [1, F], BF16, kind="ExternalInput").ap()
    OUT = nc.dram_tensor("out", [N, F], F32, kind="ExternalOutput").ap()
    with tile.TileContext(nc) as tc:
        with ExitStack() as ctx:
            _emit(ctx, tc, AT, AN, XT, WTB, BIASB, OUT)
    nc.compile()
    return nc


def get_nc():
    global _cached_nc
    if _cached_nc is None:
        _cached_nc = _build()
    return _cached_nc


def make_in_maps(node_features, adj_matrix, W, b):
    bf16 = ml_dtypes.bfloat16
    node_features = np.asarray(node_features, dtype=np.float32)
    adj_matrix = np.asarray(adj_matrix, dtype=np.float32)
    an = adj_matrix.astype(bf16)  # [B, N, N] natural
    at = np.ascontiguousarray(an.transpose(0, 2, 1))  # [B, N, N] transposed
    xt = np.ascontiguousarray(
        node_features.astype(bf16).transpose(0, 2, 1)
    )  # [B, F, N]
    wtb = np.ascontiguousarray(np.asarray(W, dtype=np.float32).T.astype(bf16))
    biasb = np.ascontiguousarray(
        np.asarray(b, dtype=np.float32).reshape(1, F).astype(bf16)
    )
    return [
        {
            "at": np.ascontiguousarray(at[c]),
            "an": np.ascontiguousarray(an[c]),
            "xt": xt[c],
            "wtb": wtb,
            "biasb": biasb,
        }
        for c in range(B)
    ]


def kernel(node_features, adj_matrix, W, b):
    nc = get_nc()
    in_maps = make_in_maps(node_features, adj_matrix, W, b)
    res = run_bass_kernel_spmd(nc, in_maps, core_ids=list(range(B)))
    return np.stack([r["out"] for r in res.results], axis=0)
